# revision 20
# baseline (speedup 1.0000x reference)
"""CapsuleLayer dynamic-routing kernel for 8 Trainium2 NeuronCores (v3).

Problem (hardcoded shapes):
  x [512, 1152, 8] f32, W [10, 1152, 8, 16] f32
  priors = einsum('bri,nrio->nbro'); 3 rounds of softmax-over-R routing.
  out [10, 512, 1, 1, 16] f32.

Sharding: 4-way batch (128 each) x 2-way capsule N (5 each) over 8 cores.

v3 design (PE all-bf16; Pool is SBUF-only on TRN2 — no PSUM operands):
  - priors: block-diag bf16 matmuls into f32 PSUM waves ([128,1024], 2 banks),
    ACT copies waves -> pa [b, o, r] bf16; round-1 uniform sum rides the PE
    (wd2 dense W shares the stationary x chunk).
  - logit updates: 16 DVE tensor_scalar muls (4x mode) into prod16, then an
    o add-tree (LU1 tree on Pool, LU2 tree on DVE; fp32 combine).
  - weighted sums (rounds 2+3) on the PE: e transposed+i-broadcast per
    r-block (lhsT = e-slice AP with a stride-0 i dim, rhs = identity) into
    bf16 PSUM chunks, plain DMA PSUM->SBUF, y = eT*xT elementwise (DVE for
    WS2 / Pool for WS3), then 16-col accumulating matmuls against wd2 give
    sT [o, b]; copy + PE-transpose back to [b, o].
  Per-iteration emission order: create(k) -> tail(k-1) -> head(k).
"""

import numpy as np

B, R, I, O, N = 512, 1152, 8, 16, 10
BG, NG = 4, 2              # batch groups x capsule groups = 8 cores
BL, NL = B // BG, N // NG  # 128, 5
RB = R // 16               # 72 r-blocks of 16
WV = 4                     # r-blocks per PSUM wave ([128, 1024] f32, 2 banks)
QRB = 18                   # r-blocks per Wbd quarter buffer
NQ = RB // QRB             # 4
CG = 8                     # r-blocks per eTrep chunk ([128, 1024] bf16, 1 bank)
NCORES = 8

_CACHE = {}


def _build_program(debug=False, repeat=1):
    import concourse.tile as tile
    from concourse import bacc, mybir

    F32 = mybir.dt.float32
    BF16 = mybir.dt.bfloat16
    ALU = mybir.AluOpType
    ACTF = mybir.ActivationFunctionType

    nc = bacc.Bacc("TRN2", target_bir_lowering=False, debug=debug,
                   num_devices=NCORES)

    xT_d = nc.dram_tensor("xT", [128, RB * BL], BF16, kind="ExternalInput")
    x2_d = nc.dram_tensor("x2", [128, I * R], BF16, kind="ExternalInput")
    wd3_d = nc.dram_tensor("wd3", [NL, 128, I * 9 * O], BF16, kind="ExternalInput")
    Wsl_d = nc.dram_tensor("Wsl", [NL, 16, 8, RB, O], BF16, kind="ExternalInput")
    eye_d = nc.dram_tensor("eye", [128, 128], BF16, kind="ExternalInput")
    eye16_d = nc.dram_tensor("eye16", [16, 16], F32, kind="ExternalInput")
    out_d = nc.dram_tensor("out", [128, NL * O], F32, kind="ExternalOutput")

    with tile.TileContext(nc) as tc:
        with (
            tc.tile_pool(name="const", bufs=1) as cpool,
            tc.tile_pool(name="psum", bufs=2, space="PSUM") as pspool,
            tc.tile_pool(name="psum1", bufs=1, space="PSUM") as pspool1,
            tc.tile_pool(name="small", bufs=4) as smpool,
        ):
            xT = cpool.tile([128, RB * BL], BF16, tag="xT")
            x2 = cpool.tile([128, I, R], BF16, tag="x2")
            wd3 = [cpool.tile([128, I, 9, O], BF16, name=f"wd3_{j}")
                   for j in range(2)]
            eye = cpool.tile([128, 128], BF16, tag="eye")
            eye16 = cpool.tile([16, 16], F32, tag="eye16")
            outacc = cpool.tile([128, NL * O], F32, tag="outacc")
            wbd = [cpool.tile([128, QRB, 16 * O], BF16, name=f"wbd{j}")
                   for j in range(2)]
            wd2 = [cpool.tile([128, RB, O], BF16, name=f"wd2_{j}")
                   for j in range(2)]
            pa = [cpool.tile([128, O, R], BF16, name=f"pa{j}") for j in range(2)]
            prod16 = cpool.tile([128, O, R], BF16, tag="prod16")
            ysb = [cpool.tile([128, CG * 128], BF16, name=f"ysb{j}")
                   for j in range(2)]
            a1t = [cpool.tile([128, R], F32, name=f"a1t{j}") for j in range(2)]
            l2 = cpool.tile([128, R], F32, tag="l2")
            e2 = [cpool.tile([128, R], BF16, name=f"e2_{j}") for j in range(2)]
            e3 = [cpool.tile([128, R], BF16, name=f"e3_{j}") for j in range(2)]
            lut = cpool.tile([128, R], BF16, tag="lut")

            nc.sync.dma_start(xT[:], xT_d[:])
            nc.sync.dma_start(x2[:], x2_d[:].rearrange("p (i r) -> p i r", i=I))
            nc.sync.dma_start(eye[:], eye_d[:])
            nc.sync.dma_start(eye16[:], eye16_d[:])
            nc.gpsimd.memset(wbd[0][:], 0.0)
            nc.gpsimd.memset(wbd[1][:], 0.0)

            def squash(su_ap, z_recip_ap, dst_v):
                s = smpool.tile([128, O], F32, tag="s")
                if z_recip_ap is None:
                    nc.vector.tensor_scalar_mul(s[:], su_ap, 1.0 / R)
                else:
                    nc.vector.tensor_scalar_mul(s[:], su_ap, z_recip_ap)
                sqj = smpool.tile([128, O], F32, tag="sqj")
                n2 = smpool.tile([128, 1], F32, tag="n2")
                nc.vector.tensor_mul(sqj[:], s[:], s[:])
                nc.vector.tensor_reduce(n2[:], sqj[:],
                                        axis=mybir.AxisListType.X,
                                        op=ALU.add)
                rt = smpool.tile([128, 1], F32, tag="rt")
                nc.scalar.activation(rt[:], n2[:], ACTF.Sqrt)
                u = smpool.tile([128, 1], F32, tag="u")
                nc.vector.tensor_scalar_add(u[:], n2[:], 1.0)
                rr = smpool.tile([128, 1], F32, tag="rr")
                nc.vector.reciprocal(rr[:], u[:])
                sc = smpool.tile([128, 1], F32, tag="sc")
                nc.vector.tensor_mul(sc[:], rt[:], rr[:])
                nc.vector.tensor_scalar_mul(dst_v, s[:], sc[:])
                return s

            def lu_stage(nr, v_ap, l_prev, l_dst, scr):
                """l_dst = (l_prev +) sum_o pan[:,o,:]*v[:,o]; products and
                the o add-tree live in scr (prod16, or pan itself for LU2
                since pan is dead afterwards)."""
                pan = pa[nr % 2]
                for o in range(O):
                    nc.vector.tensor_scalar_mul(scr[:, o, :], pan[:, o, :],
                                                v_ap[:, o:o + 1])
                nc.vector.tensor_add(scr[:, 0:8, :], scr[:, 0:8, :],
                                     scr[:, 8:16, :])
                nc.vector.tensor_add(scr[:, 0:4, :], scr[:, 0:4, :],
                                     scr[:, 4:8, :])
                nc.vector.tensor_add(scr[:, 0:2, :], scr[:, 0:2, :],
                                     scr[:, 2:4, :])
                if l_prev is None:
                    nc.vector.tensor_add(l_dst[:], scr[:, 0, :],
                                         scr[:, 1, :])
                else:
                    nc.vector.tensor_add(lut[:], scr[:, 0, :], scr[:, 1, :])
                    nc.vector.tensor_add(l_dst[:], l_prev[:], lut[:])

            def create_stage(nr, su_ps, q_lo, q_hi):
                n = nr % NL
                pan = pa[nr % 2]
                wd = wd2[nr % 2]
                if q_lo == 0:
                    nc.sync.dma_start(
                        wd[:], Wsl_d[n].rearrange("d i rb o -> (d i) rb o"))
                    nc.sync.dma_start(
                        wd3[nr % 2][:],
                        wd3_d[n].rearrange("p (i rc o) -> p i rc o", i=I, rc=9))
                for q in range(q_lo, q_hi):
                    wq = wbd[(nr * NQ + q) % 2]
                    for d in range(16):
                        nc.sync.dma_start(
                            wq[d * 8:(d + 1) * 8, :, d * O:(d + 1) * O],
                            Wsl_d[n, d, :, q * QRB:(q + 1) * QRB, :])
                    # 18 r-blocks per quarter in waves of 4,4,4,4,2
                    wl = 0
                    for wv in (4, 4, 4, 4, 2):
                        w_lo = q * QRB + wl
                        wave = pspool1.tile([128, WV * 16 * O], F32, tag="wave")
                        for k in range(wv):
                            rb = w_lo + k
                            nc.tensor.matmul(
                                wave[:, k * 256:(k + 1) * 256],
                                xT[:, rb * BL:(rb + 1) * BL],
                                wq[:, wl + k, :],
                                start=True, stop=True)
                            nc.tensor.matmul(
                                su_ps[:], xT[:, rb * BL:(rb + 1) * BL],
                                wd[:, rb, :],
                                start=(rb == 0), stop=(rb == RB - 1),
                                skip_group_check=True)
                        w0 = w_lo * 16
                        dst = pan[:, :, w0:w0 + wv * 16].rearrange(
                            "p o (rb r2) -> p o rb r2", rb=wv)
                        src = wave[:, 0:wv * 256].rearrange(
                            "p (rb r2 o) -> p o rb r2", rb=wv, r2=16, o=O)
                        nc.scalar.copy(dst, src)
                        wl += wv

            def ws_pe(nr, e_ap, s_ps, mul_pool, ptag="sm"):
                """s[b,o] = sum_r e[b,r] P[b,r,o] = sum_{r,i} W*(e*x).
                z = e*x (DVE 2x, i-major), PE transposes contiguous 128-col
                chunks of z, DVE copies them to SBUF, PE contracts against
                wd3 [p=r%128, i, rc, o]."""
                wdn = wd3[nr % 2]
                zv = prod16[:, 0:8, :]
                e3d = e_ap[:].unsqueeze(1).broadcast_to([128, I, R])
                nc.vector.tensor_mul(zv, x2[:], e3d)
                zflat = zv.rearrange("p i r -> p (i r)")
                sT_ps = pspool1.tile([16, 128], F32, tag=ptag)
                for c in range(9):
                    et = pspool.tile([128, CG * 128], BF16, tag="etr")
                    for k in range(CG):
                        col = (c * CG + k) * 128
                        nc.tensor.transpose(et[:, k * 128:(k + 1) * 128],
                                            zflat[:, col:col + 128], eye[:])
                    yb = ysb[c % 2]
                    nc.vector.tensor_copy(yb[:], et[:])
                    for k in range(CG):
                        ch = c * CG + k
                        i_, rc = divmod(ch, 9)
                        nc.tensor.matmul(
                            sT_ps[:], wdn[:, i_, rc, :],
                            yb[:, k * 128:(k + 1) * 128],
                            start=(ch == 0), stop=(ch == 71),
                            skip_group_check=True)
                sTs = smpool.tile([16, 128], F32, tag="sTs")
                nc.scalar.copy(sTs[:], sT_ps[:])
                nc.tensor.transpose(s_ps[:], sTs[:], eye16[:])

            def exp_stage(l_ap, e_dst):
                z = smpool.tile([128, 1], F32, tag="z")
                nc.scalar.activation(e_dst[:], l_ap, ACTF.Exp, accum_out=z[:])
                rz = smpool.tile([128, 1], F32, tag="rz")
                nc.vector.reciprocal(rz[:], z[:])
                return rz

            def rounds_head(nr, su_ps):
                v1 = smpool.tile([128, O], F32, tag="v1")
                squash(su_ps[:], None, v1[:])
                lu_stage(nr, v1, None, a1t[nr % 2], scr=prod16)

            def tail_a_pre(nr):
                pass

            def tail_a(nr):
                a1 = a1t[nr % 2]
                rz2 = exp_stage(a1[:], e2[nr % 2])
                s2_ps = pspool1.tile([128, O], F32, tag="sm")
                ws_pe(nr, e2[nr % 2], s2_ps, mul_pool=False)
                v2 = smpool.tile([128, O], F32, tag="v2")
                squash(s2_ps[:], rz2[:], v2[:])
                return v2

            def tail_b(nr, v2):
                a1 = a1t[nr % 2]
                lu_stage(nr, v2, a1[:], l2, scr=pa[nr % 2])
                rz3 = exp_stage(l2[:], e3[nr % 2])
                return rz3

            def tail_c(nr, rz3):
                n = nr % NL
                s3_ps = pspool1.tile([128, O], F32, tag="sm2")
                ws_pe(nr, e3[nr % 2], s3_ps, mul_pool=False, ptag="sm2")
                s3f = smpool.tile([128, O], F32, tag="s3f")
                nc.vector.tensor_scalar_mul(s3f[:], s3_ps[:], rz3[:])
                sqj = smpool.tile([128, O], F32, tag="sqj3")
                n2 = smpool.tile([128, 1], F32, tag="n23")
                nc.vector.tensor_mul(sqj[:], s3f[:], s3f[:])
                nc.vector.tensor_reduce(n2[:], sqj[:],
                                        axis=mybir.AxisListType.X,
                                        op=ALU.add)
                rt = smpool.tile([128, 1], F32, tag="rt3")
                nc.scalar.activation(rt[:], n2[:], ACTF.Sqrt)
                uu = smpool.tile([128, 1], F32, tag="u3")
                nc.vector.tensor_scalar_add(uu[:], n2[:], 1.0)
                rr = smpool.tile([128, 1], F32, tag="rr3")
                nc.vector.reciprocal(rr[:], uu[:])
                sc = smpool.tile([128, 1], F32, tag="sc3")
                nc.vector.tensor_mul(sc[:], rt[:], rr[:])
                nc.vector.tensor_scalar_mul(outacc[:, n * O:(n + 1) * O],
                                            s3f[:], sc[:])

            NT = repeat * NL
            prev = None
            for nr in range(NT):
                su_ps = pspool1.tile([128, O], F32, tag="su_ps")
                if prev is not None:
                    v2p = tail_a_pre(prev)
                create_stage(nr, su_ps, 0, 2)
                if prev is not None:
                    v2 = tail_a(prev)
                create_stage(nr, su_ps, 2, 4)
                if prev is not None:
                    rz3 = tail_b(prev, v2)
                rounds_head(nr, su_ps)
                if prev is not None:
                    tail_c(prev, rz3)
                prev = nr
            v2 = tail_a(prev)
            rz3 = tail_b(prev, v2)
            tail_c(prev, rz3)

            nc.sync.dma_start(out_d[:], outacc[:])

    nc.compile()
    return nc


def _host_prep(x, W):
    """Build per-core input maps (bf16)."""
    import ml_dtypes
    bf16 = ml_dtypes.bfloat16
    x = np.ascontiguousarray(x, dtype=np.float32)
    W = np.ascontiguousarray(W, dtype=np.float32)
    eye = np.eye(128, dtype=bf16)
    eye16 = np.eye(16, dtype=np.float32)
    in_maps = []
    for c in range(NCORES):
        bg, ng = c % BG, c // BG
        xs = x[bg * BL:(bg + 1) * BL]                      # [128, 1152, 8]
        # xT[p=(r16*8+i), rb*128+b] = xs[b, rb*16+r16, i]
        xT = np.ascontiguousarray(
            xs.reshape(BL, RB, 16, 8).transpose(2, 3, 1, 0)
            .reshape(128, RB * BL)).astype(bf16)
        Wfull = W[ng * NL:(ng + 1) * NL]                   # [NL, R, I, O]
        Wn = Wfull.reshape(NL, RB, 16, 8, O)
        # Wsl[n, d, i, rb, o] = W[n, rb*16+d, i, o]
        Wsl = np.ascontiguousarray(Wn.transpose(0, 2, 3, 1, 4)).astype(bf16)
        # x2[b, i*R + r] = xs[b, r, i]
        x2 = np.ascontiguousarray(
            xs.transpose(0, 2, 1).reshape(BL, I * R)).astype(bf16)
        # wd3[n, p, i*9*O + rc*O + o] = W[n, rc*128+p, i, o]
        wd3 = np.ascontiguousarray(
            Wfull.reshape(NL, 9, 128, I, O).transpose(0, 2, 3, 1, 4)
            .reshape(NL, 128, I * 9 * O)).astype(bf16)
        in_maps.append({"xT": xT, "x2": x2, "Wsl": Wsl, "wd3": wd3,
                        "eye": eye, "eye16": eye16})
    return in_maps


def _gather(results):
    out = np.zeros((N, B, 1, 1, O), np.float32)
    for c in range(NCORES):
        bg, ng = c % BG, c // BG
        o = results[c]["out"].reshape(BL, NL, O)           # [b, n, o]
        out[ng * NL:(ng + 1) * NL, bg * BL:(bg + 1) * BL, 0, 0, :] = \
            o.transpose(1, 0, 2)
    return out


def kernel(x, W):
    from concourse.bass_utils import run_bass_kernel_spmd
    if "nc" not in _CACHE:
        _CACHE["nc"] = _build_program()
    nc = _CACHE["nc"]
    in_maps = _host_prep(x, W)
    res = run_bass_kernel_spmd(nc, in_maps, core_ids=list(range(NCORES)))
    _CACHE["last_results"] = res
    return _gather(res.results)


if __name__ == "__main__":
    d = np.load("/root/problem/work/ref.npz")
    out = kernel(d["x"], d["W"])
    exp = d["expected"]
    rel = np.linalg.norm(out - exp) / np.linalg.norm(exp)
    print("rel err:", rel)
